# revision 18
# baseline (speedup 1.0000x reference)
"""Causal (cumulative) LayerNorm Trainium2 Bass kernel.

Full-input contract: kernel(inputs, gamma, beta) takes the full
(B=8, K=16000, H=256) f32 tensor, shards batch across 8 NeuronCores
(one sample per core), and returns the full (8, 16000, 256) output.

Per-core algorithm (x is (K, H)):
  rowsum[k]   = sum_h x[k, h]
  rowsumsq[k] = sum_h x[k, h]^2
  csum = cumsum(rowsum); cpow = cumsum(rowsumsq)
  mean[k] = csum[k] / (H*(k+1));  msq[k] = cpow[k] / (H*(k+1))
  var[k] = msq[k] - mean[k]^2
  out[k, h] = gamma[h] * (x[k, h] - mean[k]) / sqrt(var[k] + EPS) + beta[h]

v2: x and y move over HBM in bf16 (host casts f32<->bf16; the rel-err
budget is 2e-2 and end-to-end bf16 I/O measures ~7e-3), halving DMA
traffic to ~16.5 MB/core. All stats math stays f32 on-chip.

Layout: row k = b*3200 + p*25 + r for band b in 0..4, partition p in
0..127, r in 0..24. Each band is one (128, 25, 256) bf16 SBUF tile with
contiguous 12.5 KB per-partition runs in HBM.

Engine budget per band: DVE does the 25 bn_stats + scans + reciprocal +
the 5 per-group gamma multiplies (big-tile tensor_tensor, cheap on DVE,
5x cheaper than gpsimd). ACT does sqrt + 15/25 affine rows. GPSIMD does
the mean/var/nmi chain + 10/25 affine rows. PE does the even/odd stat
merges and the cross-partition prefix (strictly-triangular matmul).
SP (sync) issues all load and store DMA triggers. out_band emits all
affines first, then gammas+stores, so no engine queue head-blocks on a
cross-engine dependency.
"""

import numpy as np
import ml_dtypes

import concourse.bass as bass
import concourse.bacc as bacc
import concourse.tile as tile
from concourse import mybir
from concourse.bass_utils import run_bass_kernel_spmd

EPS = 1e-8
B, K, H = 8, 16000, 256
P = 128                  # SBUF partitions = chunks per band
CL = 25                  # rows per chunk (per partition per band)
BANDS = K // (P * CL)    # 5
G = 5                    # rows per gamma/store group
NGB = CL // G            # 5 groups per band
F32 = mybir.dt.float32
BF16 = mybir.dt.bfloat16
ALU = mybir.AluOpType
ACTF = mybir.ActivationFunctionType

# per-group affine engine: "act" or "gps"
AFF_ENG = ["act", "gps", "act", "gps", "act"]


def _build(use_beta: bool):
    nc = bacc.Bacc("TRN2", target_bir_lowering=False, debug=False)

    x = nc.declare_dram_parameter("x", [K, H], BF16, isOutput=False)
    gamma_b = nc.declare_dram_parameter("gamma_b", [P, H], BF16, isOutput=False)
    beta_b = (
        nc.declare_dram_parameter("beta_b", [P, H], BF16, isOutput=False)
        if use_beta
        else None
    )
    utri = nc.declare_dram_parameter("utri", [P, P], F32, isOutput=False)
    ident = nc.declare_dram_parameter("ident", [P, P], F32, isOutput=False)
    ident_sc = nc.declare_dram_parameter("ident_sc", [P, P], F32, isOutput=False)
    ones_col = nc.declare_dram_parameter("ones_col", [P, 1], F32, isOutput=False)
    ones_row = nc.declare_dram_parameter("ones_row", [1, P], F32, isOutput=False)
    invc_m = nc.declare_dram_parameter("invc_m", [P, BANDS, CL], F32, isOutput=False)
    invc_p = nc.declare_dram_parameter("invc_p", [P, BANDS, CL], F32, isOutput=False)
    y = nc.declare_dram_parameter("y", [K, H], BF16, isOutput=True)

    xr = x.rearrange("(b p r) h -> b p r h", p=P, r=CL)   # [5, 128, 25, 256]
    yr = y.rearrange("(b p r) h -> b p r h", p=P, r=CL)

    with tile.TileContext(nc) as tc:
        with (
            tc.tile_pool(name="singles", bufs=1) as singles,
            tc.tile_pool(name="xband", bufs=1) as xband,
            tc.tile_pool(name="opool", bufs=12) as opool,
            tc.tile_pool(name="segp", bufs=3) as segp,
            tc.tile_pool(name="psum", bufs=2, space="PSUM") as psum,
        ):
            # trigger ALL band loads up front (each band has its own buffer,
            # so there is no reuse hazard), with band 0's chunks first so
            # bn_stats starts ASAP; const loads go between band-0 and the
            # rest (nothing reads them in the first ~8us).
            xband_tiles = []
            band_bounds = []
            for b in range(BANDS):
                xband_tiles.append(
                    xband.tile([P, CL, H], BF16, name=f"xt{b}", tag=f"xt{b}")
                )
                band_bounds.append(
                    [0, 2, 7, 13, 19, 25] if b == 0 else [0, 13, 25]
                )

            def trigger_band(b):
                xt = xband_tiles[b]
                xv = xr[b]
                for u in range(len(band_bounds[b]) - 1):
                    lo, hi = band_bounds[b][u], band_bounds[b][u + 1]
                    nc.sync.dma_start(out=xt[:, lo:hi, :], in_=xv[:, lo:hi, :])

            trigger_band(0)
            sb_utri = singles.tile([P, P], F32)
            nc.sync.dma_start(out=sb_utri[:], in_=utri[:])
            sb_ident = singles.tile([P, P], F32)
            nc.sync.dma_start(out=sb_ident[:], in_=ident[:])
            sb_identsc = singles.tile([P, P], F32)
            nc.sync.dma_start(out=sb_identsc[:], in_=ident_sc[:])
            trigger_band(1)
            sb_invm = singles.tile([P, BANDS, CL], F32)
            nc.sync.dma_start(out=sb_invm[:], in_=invc_m[:])
            sb_invp = singles.tile([P, BANDS, CL], F32)
            nc.sync.dma_start(out=sb_invp[:], in_=invc_p[:])
            sb_gamma = singles.tile([P, H], BF16)
            nc.sync.dma_start(out=sb_gamma[:], in_=gamma_b[:])
            if use_beta:
                sb_beta = singles.tile([P, H], BF16)
                nc.sync.dma_start(out=sb_beta[:], in_=beta_b[:])
            for b in range(2, BANDS):
                trigger_band(b)
            sb_onec = singles.tile([P, 1], F32)
            nc.sync.dma_start(out=sb_onec[:], in_=ones_col[:])
            sb_oner = singles.tile([1, P], F32)
            nc.sync.dma_start(out=sb_oner[:], in_=ones_row[:])

            sb_eps = singles.tile([P, 1], F32)
            nc.vector.memset(sb_eps[:], EPS)
            carry = singles.tile([1, 2], F32)
            nc.vector.memset(carry[:], 0.0)

            gamma_bc = sb_gamma[:].rearrange("p (o h) -> p o h", o=1).to_broadcast(
                (P, G, H)
            )
            if use_beta:
                beta_bc = sb_beta[:].rearrange("p (o h) -> p o h", o=1).to_broadcast(
                    (P, G, H)
                )

            xb = []
            invb = {}
            nmib = {}

            def load_band(b):
                xt = xband_tiles[b]
                bnb = segp.tile([P, CL, 6], F32, tag="bn")
                for r in range(CL):
                    nc.vector.bn_stats(out=bnb[:, r, :], in_=xt[:, r, :])
                xb.append(xt)
                return bnb

            def scan_band(b, bnb):
                me = bnb[:, :, 1]
                mo = bnb[:, :, 4]
                m2e = bnb[:, :, 2]
                m2o = bnb[:, :, 5]
                # rowsum/128: merge even/odd on the PE via identity-matmul
                # accumulation (I@me + I@mo); rowsumsq picks up
                # m2e + m2o + 128*(me^2 + mo^2) in one PSUM accumulation,
                # with the 128x scale folded into the (128*I) lhsT so the
                # squares are plain gpsimd tensor_tensors.
                pe = segp.tile([P, CL], F32, tag="pe")
                nc.vector.tensor_mul(out=pe[:], in0=me, in1=me)
                po = segp.tile([P, CL], F32, tag="po")
                nc.vector.tensor_mul(out=po[:], in0=mo, in1=mo)
                se_ps = psum.tile([P, CL], F32, tag="se_ps")
                nc.tensor.matmul(
                    se_ps[:], lhsT=sb_ident[:], rhs=me, start=True, stop=False
                )
                nc.tensor.matmul(
                    se_ps[:], lhsT=sb_ident[:], rhs=mo, start=False, stop=True
                )
                sp_ps = psum.tile([P, CL], F32, tag="sp_ps")
                nc.tensor.matmul(
                    sp_ps[:], lhsT=sb_ident[:], rhs=m2e, start=True, stop=False
                )
                nc.tensor.matmul(
                    sp_ps[:], lhsT=sb_ident[:], rhs=m2o, start=False, stop=False
                )
                nc.tensor.matmul(
                    sp_ps[:], lhsT=sb_identsc[:], rhs=pe[:], start=False, stop=False
                )
                nc.tensor.matmul(
                    sp_ps[:], lhsT=sb_identsc[:], rhs=po[:], start=False, stop=True
                )

                # prefix along r within each chunk
                scan_s = segp.tile([P, CL], F32, tag="scan_s")
                nc.vector.tensor_tensor_scan(
                    out=scan_s[:], data0=se_ps[:], data1=pe[:],
                    initial=0.0, op0=ALU.add, op1=ALU.bypass,
                )
                scan_p = segp.tile([P, CL], F32, tag="scan_p")
                nc.vector.tensor_tensor_scan(
                    out=scan_p[:], data0=sp_ps[:], data1=pe[:],
                    initial=0.0, op0=ALU.add, op1=ALU.bypass,
                )

                # chunk totals -> exclusive prefix across partitions (PE)
                tot = segp.tile([P, 2], F32, tag="tot")
                nc.vector.tensor_copy(out=tot[:, 0:1], in_=scan_s[:, CL - 1:CL])
                nc.vector.tensor_copy(out=tot[:, 1:2], in_=scan_p[:, CL - 1:CL])
                offs = psum.tile([P, 2], F32, tag="offs")
                nc.tensor.matmul(
                    offs[:], lhsT=sb_utri[:], rhs=tot[:], start=True, stop=False
                )
                nc.tensor.matmul(
                    offs[:], lhsT=sb_oner[:], rhs=carry[:], start=False, stop=True
                )
                # band total (1,2) for the running carry
                btot = psum.tile([1, 2], F32, tag="btot")
                nc.tensor.matmul(
                    btot[:], lhsT=sb_onec[:], rhs=tot[:], start=True, stop=True
                )
                nc.vector.tensor_add(out=carry[:], in0=carry[:], in1=btot[:])

                # mean / msq on DVE (AP-scalar stt is DVE-only); var on
                # gpsimd; sqrt ACT; rstd / -mean*rstd back on DVE
                mean_c = segp.tile([P, CL], F32, tag="mean_c")
                nc.vector.scalar_tensor_tensor(
                    out=mean_c[:], in0=scan_s[:], scalar=offs[:, 0:1],
                    in1=sb_invm[:, b, :], op0=ALU.add, op1=ALU.mult,
                )
                msq_c = segp.tile([P, CL], F32, tag="msq_c")
                nc.vector.scalar_tensor_tensor(
                    out=msq_c[:], in0=scan_p[:], scalar=offs[:, 1:2],
                    in1=sb_invp[:, b, :], op0=ALU.add, op1=ALU.mult,
                )
                var_c = segp.tile([P, CL], F32, tag="var_c")
                nc.gpsimd.tensor_mul(out=var_c[:], in0=mean_c[:], in1=mean_c[:])
                nc.gpsimd.tensor_sub(out=var_c[:], in0=msq_c[:], in1=var_c[:])
                sd_c = segp.tile([P, CL], F32, tag="sd_c")
                nc.scalar.activation(
                    out=sd_c[:], in_=var_c[:], func=ACTF.Sqrt, bias=sb_eps[:],
                )
                inv_c = segp.tile([P, CL], F32, tag="inv_c")
                nc.vector.reciprocal(out=inv_c[:], in_=sd_c[:])
                nmi_c = segp.tile([P, CL], F32, tag="nmi_c")
                nc.vector.scalar_tensor_tensor(
                    out=nmi_c[:], in0=mean_c[:], scalar=-1.0, in1=inv_c[:],
                    op0=ALU.mult, op1=ALU.mult,
                )
                invb[b] = inv_c
                nmib[b] = nmi_c

            def out_band(b):
                # all affines first (ACT and GPS groups run concurrently),
                # then per-group gamma (DVE) + store trigger (SP), so no
                # queue head-blocks on a cross-engine dependency.
                xt = xb[b]
                inv_c = invb[b]
                nmi_c = nmib[b]
                obs = []
                for j in range(NGB):
                    ob = opool.tile([P, G, H], BF16)
                    for jr in range(G):
                        r = j * G + jr
                        if AFF_ENG[j] == "act":
                            nc.scalar.activation(
                                out=ob[:, jr, :], in_=xt[:, r, :],
                                func=ACTF.Identity,
                                bias=nmi_c[:, r:r + 1], scale=inv_c[:, r:r + 1],
                            )
                        else:
                            nc.gpsimd.tensor_scalar(
                                out=ob[:, jr, :], in0=xt[:, r, :],
                                scalar1=inv_c[:, r:r + 1],
                                scalar2=nmi_c[:, r:r + 1],
                                op0=ALU.mult, op1=ALU.add,
                            )
                    obs.append(ob)
                for j, ob in enumerate(obs):
                    nc.vector.tensor_mul(out=ob[:], in0=ob[:], in1=gamma_bc)
                    if use_beta:
                        nc.vector.tensor_add(out=ob[:], in0=ob[:], in1=beta_bc)
                    # store triggers on SP: every load trigger is already
                    # queued ahead of them, so their gamma sem-waits can
                    # never delay a load
                    nc.sync.dma_start(
                        out=yr[b][:, j * G:(j + 1) * G, :], in_=ob[:],
                    )

            bn0 = load_band(0)
            scan_band(0, bn0)
            for b in range(1, BANDS):
                bnb = load_band(b)
                out_band(b - 1)
                scan_band(b, bnb)
            out_band(BANDS - 1)

    nc.compile()
    return nc


_CACHE = {}


def _get(use_beta: bool):
    if use_beta not in _CACHE:
        _CACHE[use_beta] = _build(use_beta)
    return _CACHE[use_beta]


def _make_consts():
    # strictly-upper triangular ones: lhsT[q, p] = 1 iff q < p
    utri = np.triu(np.ones((P, P), dtype=np.float32), k=1)
    ident = np.eye(P, dtype=np.float32)
    ident_sc = np.eye(P, dtype=np.float32) * 128.0
    ones_col = np.ones((P, 1), dtype=np.float32)
    ones_row = np.ones((1, P), dtype=np.float32)
    k = np.arange(K, dtype=np.float64).reshape(BANDS, P, CL)  # [b, p, r]
    counts = np.transpose(k, (1, 0, 2)) + 1.0                 # [p, b, r]
    invc_m = (1.0 / (2.0 * counts)).astype(np.float32)
    invc_p = (1.0 / (float(H) * counts)).astype(np.float32)
    return utri, ident, ident_sc, ones_col, ones_row, invc_m, invc_p


def _prepare(inputs, gamma, beta):
    inputs = np.asarray(inputs, dtype=np.float32)
    gamma = np.asarray(gamma, dtype=np.float32).reshape(1, H)
    beta = np.asarray(beta, dtype=np.float32).reshape(1, H)
    use_beta = bool(np.any(beta))

    x_bf = inputs.astype(ml_dtypes.bfloat16)
    gamma_b = np.ascontiguousarray(
        np.broadcast_to(gamma, (P, H)).astype(ml_dtypes.bfloat16)
    )
    utri, ident, ident_sc, ones_col, ones_row, invc_m, invc_p = _make_consts()

    in_maps = []
    for b in range(B):
        m = {
            "x": np.ascontiguousarray(x_bf[b]),
            "gamma_b": gamma_b,
            "utri": utri,
            "ident": ident,
            "ident_sc": ident_sc,
            "ones_col": ones_col,
            "ones_row": ones_row,
            "invc_m": invc_m,
            "invc_p": invc_p,
        }
        if use_beta:
            m["beta_b"] = np.ascontiguousarray(
                np.broadcast_to(beta, (P, H)).astype(ml_dtypes.bfloat16)
            )
        in_maps.append(m)
    return use_beta, in_maps


def kernel(inputs: np.ndarray, gamma: np.ndarray, beta: np.ndarray) -> np.ndarray:
    use_beta, in_maps = _prepare(inputs, gamma, beta)
    nc = _get(use_beta)
    res = run_bass_kernel_spmd(nc, in_maps, list(range(B)))
    out = np.stack(
        [np.asarray(res.results[b]["y"]).astype(np.float32) for b in range(B)],
        axis=0,
    )
    return out


# revision 19
# speedup vs baseline: 1.0755x; 1.0755x over previous
"""Causal (cumulative) LayerNorm Trainium2 Bass kernel.

Full-input contract: kernel(inputs, gamma, beta) takes the full
(B=8, K=16000, H=256) f32 tensor, shards batch across 8 NeuronCores
(one sample per core), and returns the full (8, 16000, 256) output.

Per-core algorithm (x is (K, H)):
  rowsum[k]   = sum_h x[k, h]
  rowsumsq[k] = sum_h x[k, h]^2
  csum = cumsum(rowsum); cpow = cumsum(rowsumsq)
  mean[k] = csum[k] / (H*(k+1));  msq[k] = cpow[k] / (H*(k+1))
  var[k] = msq[k] - mean[k]^2
  out[k, h] = gamma[h] * ((x[k, h] - mean[k]) / sqrt(var + EPS)) + beta[h]

v3: x and y move over HBM in bf16 (host casts f32<->bf16; rel-err
budget is 2e-2, end-to-end bf16 I/O measures ~7e-3). The host also
PAIR-INTERLEAVES rows: band row pair (2j, 2j+1) is stored element-
interleaved (A0 B0 A1 B1 ...) in one 1 KB run, so a single 512-elem
bn_stats yields BOTH rows' stats (even lanes = row A, odd = row B):
13 bn_stats per band instead of 25, and the means/M2s come out in row
order directly - the whole PE even/odd merge (6 matmuls/band) is gone.
Row 25 of each 26-row padded chunk is zeros and never consumed.

Layout: row k = b*3200 + p*25 + r for band b in 0..4, partition p in
0..127, r in 0..24 (pad r=25). Per band per partition: 13 interleaved
pairs x 1 KB, contiguous in HBM.

Engines: DVE = bn_stats, squares, scans, mean/msq/nmi, reciprocal, and
the per-group gamma multiplies (the only engine with fast broadcasted
tensor_tensor). ACT = sqrt + 15/25 affine rows (per-row scale+bias).
GPSIMD = var + 10/25 affine rows. PE = cross-partition exclusive
prefix via strictly-triangular matmul. SP = all DMA triggers, loads
first so store sem-waits can never delay a load.
"""

import numpy as np
import ml_dtypes

import concourse.bass as bass
import concourse.bacc as bacc
import concourse.tile as tile
from concourse import mybir
from concourse.bass_utils import run_bass_kernel_spmd

EPS = 1e-8
B, K, H = 8, 16000, 256
P = 128                  # SBUF partitions = chunks per band
CL = 25                  # real rows per chunk (per partition per band)
CLP = 26                 # padded rows (13 pairs)
NPAIR = 13
BANDS = K // (P * CL)    # 5
G = 5                    # rows per gamma/store group
NGB = CL // G            # 5 groups per band
WPB = NPAIR * 2 * H      # interleaved elems per partition per band (6656)
F32 = mybir.dt.float32
BF16 = mybir.dt.bfloat16
ALU = mybir.AluOpType
ACTF = mybir.ActivationFunctionType

# per-group affine engine: "act" or "gps"
AFF_ENG = ["act", "gps", "act", "gps", "act"]


def _build(use_beta: bool):
    nc = bacc.Bacc("TRN2", target_bir_lowering=False, debug=False)

    x = nc.declare_dram_parameter("x", [BANDS, P, WPB], BF16, isOutput=False)
    gamma_b = nc.declare_dram_parameter("gamma_b", [P, H], BF16, isOutput=False)
    beta_b = (
        nc.declare_dram_parameter("beta_b", [P, H], BF16, isOutput=False)
        if use_beta
        else None
    )
    utri = nc.declare_dram_parameter("utri", [P, P], F32, isOutput=False)
    ones_col = nc.declare_dram_parameter("ones_col", [P, 1], F32, isOutput=False)
    ones_row = nc.declare_dram_parameter("ones_row", [1, P], F32, isOutput=False)
    invc = nc.declare_dram_parameter("invc", [P, BANDS, CLP], F32, isOutput=False)
    y = nc.declare_dram_parameter("y", [K, H], BF16, isOutput=True)

    yr = y.rearrange("(b p r) h -> b p r h", p=P, r=CL)

    with tile.TileContext(nc) as tc:
        with (
            tc.tile_pool(name="singles", bufs=1) as singles,
            tc.tile_pool(name="xband", bufs=1) as xband,
            tc.tile_pool(name="opool", bufs=12) as opool,
            tc.tile_pool(name="segp", bufs=3) as segp,
            tc.tile_pool(name="psum", bufs=2, space="PSUM") as psum,
        ):
            # trigger ALL band loads up front (per-band buffers, no reuse
            # hazard); band 0 in small chunks so bn_stats starts ASAP;
            # const loads slot in after band 0 (unused for the first ~8us)
            xband_tiles = [
                xband.tile([P, NPAIR, 2 * H], BF16, name=f"xt{b}", tag=f"xt{b}")
                for b in range(BANDS)
            ]
            band_bounds = [
                [0, 1, 3, 6, 9, NPAIR] if b == 0 else [0, 7, NPAIR]
                for b in range(BANDS)
            ]

            def trigger_band(b):
                xt = xband_tiles[b]
                xv = x[b].rearrange("p (j w) -> p j w", j=NPAIR)
                for u in range(len(band_bounds[b]) - 1):
                    lo, hi = band_bounds[b][u], band_bounds[b][u + 1]
                    nc.sync.dma_start(out=xt[:, lo:hi, :], in_=xv[:, lo:hi, :])

            trigger_band(0)
            sb_utri = singles.tile([P, P], F32)
            nc.sync.dma_start(out=sb_utri[:], in_=utri[:])
            sb_invc = singles.tile([P, BANDS, CLP], F32)
            nc.sync.dma_start(out=sb_invc[:], in_=invc[:])
            trigger_band(1)
            sb_gamma = singles.tile([P, H], BF16)
            nc.sync.dma_start(out=sb_gamma[:], in_=gamma_b[:])
            if use_beta:
                sb_beta = singles.tile([P, H], BF16)
                nc.sync.dma_start(out=sb_beta[:], in_=beta_b[:])
            for b in range(2, BANDS):
                trigger_band(b)
            sb_onec = singles.tile([P, 1], F32)
            nc.sync.dma_start(out=sb_onec[:], in_=ones_col[:])
            sb_oner = singles.tile([1, P], F32)
            nc.sync.dma_start(out=sb_oner[:], in_=ones_row[:])

            sb_eps = singles.tile([P, 1], F32)
            nc.vector.memset(sb_eps[:], EPS)
            carry = singles.tile([1, 2], F32)
            nc.vector.memset(carry[:], 0.0)

            gamma_bc = sb_gamma[:].rearrange("p (o h) -> p o h", o=1).to_broadcast(
                (P, G, H)
            )
            if use_beta:
                beta_bc = sb_beta[:].rearrange("p (o h) -> p o h", o=1).to_broadcast(
                    (P, G, H)
                )

            xb = []
            invb = {}
            nmib = {}

            def load_band(b):
                xt = xband_tiles[b]
                bnb = segp.tile([P, NPAIR, 6], F32, tag="bn")
                for j in range(NPAIR):
                    nc.vector.bn_stats(out=bnb[:, j, :], in_=xt[:, j, :])
                xb.append(xt)
                return bnb

            def scan_band(b, bnb):
                # bnb cols: [ce, me, m2e, co, mo, m2o]; row 2j -> even slot,
                # row 2j+1 -> odd slot. View (j, slot) pairs in row order.
                bv = bnb[:].rearrange("p j (u v) -> p v j u", u=2, v=3)
                me_r = bv[:, 1, :, :]   # [P, 13, 2] means, row order
                m2_r = bv[:, 2, :, :]   # [P, 13, 2] M2s, row order

                # rowsum/H directly = per-row mean; rowsumsq/H via
                # M2/H + mean^2
                se_c = segp.tile([P, CLP], F32, tag="se_c")
                nc.vector.tensor_copy(out=se_c[:], in_=me_r)
                pe = segp.tile([P, CLP], F32, tag="pe")
                nc.vector.tensor_mul(out=pe[:], in0=me_r, in1=me_r)
                sp_c = segp.tile([P, CLP], F32, tag="sp_c")
                nc.vector.scalar_tensor_tensor(
                    out=sp_c[:], in0=m2_r, scalar=1.0 / H, in1=pe[:],
                    op0=ALU.mult, op1=ALU.add,
                )

                # prefix along r within each chunk
                scan_s = segp.tile([P, CLP], F32, tag="scan_s")
                nc.vector.tensor_tensor_scan(
                    out=scan_s[:], data0=se_c[:], data1=se_c[:],
                    initial=0.0, op0=ALU.add, op1=ALU.bypass,
                )
                scan_p = segp.tile([P, CLP], F32, tag="scan_p")
                nc.vector.tensor_tensor_scan(
                    out=scan_p[:], data0=sp_c[:], data1=sp_c[:],
                    initial=0.0, op0=ALU.add, op1=ALU.bypass,
                )

                # chunk totals (last REAL row = CL-1) -> exclusive prefix
                # across partitions (PE), plus running inter-band carry
                tot = segp.tile([P, 2], F32, tag="tot")
                nc.vector.tensor_copy(out=tot[:, 0:1], in_=scan_s[:, CL - 1:CL])
                nc.vector.tensor_copy(out=tot[:, 1:2], in_=scan_p[:, CL - 1:CL])
                offs = psum.tile([P, 2], F32, tag="offs")
                nc.tensor.matmul(
                    offs[:], lhsT=sb_utri[:], rhs=tot[:], start=True, stop=False
                )
                nc.tensor.matmul(
                    offs[:], lhsT=sb_oner[:], rhs=carry[:], start=False, stop=True
                )
                btot = psum.tile([1, 2], F32, tag="btot")
                nc.tensor.matmul(
                    btot[:], lhsT=sb_onec[:], rhs=tot[:], start=True, stop=True
                )
                nc.vector.tensor_add(out=carry[:], in0=carry[:], in1=btot[:])

                # mean / msq (DVE, AP-scalar from PSUM); var (gpsimd);
                # sqrt (ACT); rstd, -mean*rstd (DVE)
                mean_c = segp.tile([P, CLP], F32, tag="mean_c")
                nc.vector.scalar_tensor_tensor(
                    out=mean_c[:], in0=scan_s[:], scalar=offs[:, 0:1],
                    in1=sb_invc[:, b, :], op0=ALU.add, op1=ALU.mult,
                )
                msq_c = segp.tile([P, CLP], F32, tag="msq_c")
                nc.vector.scalar_tensor_tensor(
                    out=msq_c[:], in0=scan_p[:], scalar=offs[:, 1:2],
                    in1=sb_invc[:, b, :], op0=ALU.add, op1=ALU.mult,
                )
                var_c = segp.tile([P, CLP], F32, tag="var_c")
                nc.gpsimd.tensor_mul(out=var_c[:], in0=mean_c[:], in1=mean_c[:])
                nc.gpsimd.tensor_sub(out=var_c[:], in0=msq_c[:], in1=var_c[:])
                sd_c = segp.tile([P, CLP], F32, tag="sd_c")
                nc.scalar.activation(
                    out=sd_c[:], in_=var_c[:], func=ACTF.Sqrt, bias=sb_eps[:],
                )
                inv_c = segp.tile([P, CLP], F32, tag="inv_c")
                nc.vector.reciprocal(out=inv_c[:], in_=sd_c[:])
                nmi_c = segp.tile([P, CLP], F32, tag="nmi_c")
                nc.vector.scalar_tensor_tensor(
                    out=nmi_c[:], in0=mean_c[:], scalar=-1.0, in1=inv_c[:],
                    op0=ALU.mult, op1=ALU.mult,
                )
                invb[b] = inv_c
                nmib[b] = nmi_c

            def out_band(b):
                # all affines first (ACT and GPS groups run concurrently),
                # then per-group gamma (DVE) + store trigger (SP)
                xt = xb[b]
                xv = xt[:].rearrange("p j (i s) -> p j s i", s=2)
                inv_c = invb[b]
                nmi_c = nmib[b]
                obs = []
                for j in range(NGB):
                    ob = opool.tile([P, G, H], BF16)
                    for jr in range(G):
                        r = j * G + jr
                        xrow = xv[:, r // 2, r % 2, :]
                        if AFF_ENG[j] == "act":
                            nc.scalar.activation(
                                out=ob[:, jr, :], in_=xrow,
                                func=ACTF.Identity,
                                bias=nmi_c[:, r:r + 1], scale=inv_c[:, r:r + 1],
                            )
                        else:
                            nc.gpsimd.tensor_scalar(
                                out=ob[:, jr, :], in0=xrow,
                                scalar1=inv_c[:, r:r + 1],
                                scalar2=nmi_c[:, r:r + 1],
                                op0=ALU.mult, op1=ALU.add,
                            )
                    obs.append(ob)
                for j, ob in enumerate(obs):
                    nc.vector.tensor_mul(out=ob[:], in0=ob[:], in1=gamma_bc)
                    if use_beta:
                        nc.vector.tensor_add(out=ob[:], in0=ob[:], in1=beta_bc)
                    nc.sync.dma_start(
                        out=yr[b][:, j * G:(j + 1) * G, :], in_=ob[:],
                    )

            bn0 = load_band(0)
            scan_band(0, bn0)
            for b in range(1, BANDS):
                bnb = load_band(b)
                out_band(b - 1)
                scan_band(b, bnb)
            out_band(BANDS - 1)

    nc.compile()
    return nc


_CACHE = {}


def _get(use_beta: bool):
    if use_beta not in _CACHE:
        _CACHE[use_beta] = _build(use_beta)
    return _CACHE[use_beta]


def _make_consts():
    # strictly-upper triangular ones: lhsT[q, p] = 1 iff q < p
    utri = np.triu(np.ones((P, P), dtype=np.float32), k=1)
    ones_col = np.ones((P, 1), dtype=np.float32)
    ones_row = np.ones((1, P), dtype=np.float32)
    k = np.arange(K, dtype=np.float64).reshape(BANDS, P, CL)  # [b, p, r]
    counts = np.transpose(k, (1, 0, 2)) + 1.0                 # [p, b, r]
    invc = np.ones((P, BANDS, CLP), dtype=np.float32)         # pad row -> 1.0
    invc[:, :, :CL] = (1.0 / counts).astype(np.float32)
    return utri, ones_col, ones_row, invc


def _prep_x(x_core_bf):
    # (K, H) bf16 -> banded, 26-row padded, pair-interleaved
    xb = x_core_bf.reshape(BANDS, P, CL, H)
    xp = np.concatenate(
        [xb, np.zeros((BANDS, P, 1, H), dtype=x_core_bf.dtype)], axis=2
    )                                        # (5,128,26,256)
    xp = xp.reshape(BANDS, P, NPAIR, 2, H)   # pair j = rows (2j, 2j+1)
    xp = np.swapaxes(xp, 3, 4)               # (..., 256, 2): interleave
    return np.ascontiguousarray(xp.reshape(BANDS, P, WPB))


def _prepare(inputs, gamma, beta):
    inputs = np.asarray(inputs, dtype=np.float32)
    gamma = np.asarray(gamma, dtype=np.float32).reshape(1, H)
    beta = np.asarray(beta, dtype=np.float32).reshape(1, H)
    use_beta = bool(np.any(beta))

    x_bf = inputs.astype(ml_dtypes.bfloat16)
    gamma_b = np.ascontiguousarray(
        np.broadcast_to(gamma, (P, H)).astype(ml_dtypes.bfloat16)
    )
    utri, ones_col, ones_row, invc = _make_consts()

    in_maps = []
    for b in range(B):
        m = {
            "x": _prep_x(x_bf[b]),
            "gamma_b": gamma_b,
            "utri": utri,
            "ones_col": ones_col,
            "ones_row": ones_row,
            "invc": invc,
        }
        if use_beta:
            m["beta_b"] = np.ascontiguousarray(
                np.broadcast_to(beta, (P, H)).astype(ml_dtypes.bfloat16)
            )
        in_maps.append(m)
    return use_beta, in_maps


def kernel(inputs: np.ndarray, gamma: np.ndarray, beta: np.ndarray) -> np.ndarray:
    use_beta, in_maps = _prepare(inputs, gamma, beta)
    nc = _get(use_beta)
    res = run_bass_kernel_spmd(nc, in_maps, list(range(B)))
    out = np.stack(
        [np.asarray(res.results[b]["y"]).astype(np.float32) for b in range(B)],
        axis=0,
    )
    return out
